# revision 24
# baseline (speedup 1.0000x reference)
"""Trainium2 Bass kernel for a DoReFa-quantized ResNet BasicBlock.

    out = qact(bn2(conv3x3(qact(bn1(conv3x3(x, qw(w1)))), qw(w2*mask))) + x)

Full inputs: x (64,128,28,28) f32, w1/w2/mask2 (128,128,3,3), BN params (128,).
Data-parallel over 8 NeuronCores (8 images each); BN batch statistics are
exchanged with two tiny AllReduce collectives.

Numerical scheme (validated against the jax reference, ~1.1e-2 rel-L2 vs the
2e-2 gate; inputs are deterministic):
 - DoReFa weights quantize onto the grid m/15, m an odd integer in [-15,15].
   The integers m are computed on host (cheap, and bit-matches jax's rounding)
   and shipped as integer-valued fp16/bf16 tensors (exactly representable).
 - conv1 runs on x rounded to fp16 (single pass). The products int4 x fp16
   are exact in fp32 PSUM; the only error is the fp16 rounding of x. A bf16
   hi+lo two-pass conv1 (1.5e-3) was measured SLOWER end-to-end: all cores
   must issue the BN1-stats collective trigger before the ncfw warmup chain
   completes (~90-105us wall incl. launch skew), which the ~62us hi/lo conv1
   misses but the ~32us fp16 conv1 makes comfortably.
 - conv2's input activations are quantized to j/15, j in 0..15. Feeding the
   integers j as bf16 makes conv2 an exact integer matmul (products <= 225,
   sums <= 1152*225 < 2^24: exact in fp32 PSUM) at full bf16 PE throughput.
 - 3x3 conv = 9 shifted [128 x 128] matmuls accumulated in PSUM over a
   zero-padded [C=128 part, img, 31, 30] SBUF image layout. Each tap's moving
   operand is a fully CONTIGUOUS 420-element run (14 rows x 30 incl. 2 junk
   columns/row); junk columns land in unused PSUM columns.
 - the activation quantizer (clip / x15 / round-to-nearest-even via the
   (t + 2^23) - 2^23 trick / rescale) is one fused custom Vector-engine op;
   the residual variant also folds in the skip-connection add.
 - BN statistics: DVE bn_stats/bn_aggr per chunk -> per-core (mean, E[y^2])
   in a [P,2] tile -> PE-transpose to [2,P] (a [P,2]-shaped DMA moves 8 bytes
   per partition = 128 packets = ~4us packet-rate-bound; the transposed
   [2,P] layout is 2 big packets, ~1us) -> 1KB AllReduce(add) -> DMA back to
   a 2-partition tile -> PE-transpose back into PSUM -> rsqrt via ACT sqrt +
   DVE reciprocal + 1 Newton step (ACT's Rsqrt is blocked for accuracy). A
   junk ACT Sqrt during startup pre-loads the Sqrt table so the ~1.3us
   ACT_TABLE_LOAD stays off the post-collective critical path. The trigger
   DMAs ride the sync queue: the gpsimd queue sits blocked behind the warmup
   collective and was measured to delay the stats AllReduce by ~15us.
 - a throwaway AllGather issued at kernel start absorbs the ncfw
   first-collective setup cost (~75us) in parallel with input DMA + conv1;
   its output store is deferred to the end of the program so no engine queue
   blocks on the collective's completion.
"""

import os
import sys

import numpy as np

for _p in ("/opt/trn_rl_repo",):
    if _p not in sys.path and os.path.isdir(_p):
        sys.path.insert(0, _p)

import ml_dtypes  # noqa: E402

from concourse import bacc, mybir, tile  # noqa: E402
from concourse import bass_utils  # noqa: E402
from concourse import dve_ops  # noqa: E402
from concourse.dve_spec import C0, C1, C2, Spec, Src0, Src1, lower, minn, relu  # noqa: E402
from concourse.dve_spec import _has_src1 as has_src1  # noqa: E402
from concourse.dve_uop import DveOpSpec  # noqa: E402


def _register_dve_op(name, spec):
    for op in dve_ops.OPS:
        if op.name == name:
            return op
    row = dve_ops._CUSTOM_DVE_ROW_BASE + len(dve_ops.OPS)
    assert row < 0x20
    shas = {}
    for ver in ("v3", "v4"):
        shas[ver] = DveOpSpec(
            name=name, opcode=row, uops=lower(spec, ver=ver), rd1_en=has_src1(spec)
        ).sha(ver)
    op = dve_ops.DveOp(name, spec, subdim=False, uops_sha=shas)
    dve_ops.OPS.append(op)
    dve_ops.CUSTOM_DVE_SPECS[name] = spec
    dve_ops._SUB_OPCODE_FOR_NAME[name] = row
    return op


def _q(t, s0, s1, imm2):
    f = np.float32
    t = np.minimum(np.maximum(t, f(0.0)), f(s0)).astype(np.float32)
    t = (t + f(s1)).astype(np.float32)
    t = (t - f(s1)).astype(np.float32)
    return (t * f(imm2)).astype(np.float32)


# out = (min(relu(in*C0), C0) + C1 - C1) * C2 : with C0=15, C1=2^23,
# C2 in {1, 1/15} this is the whole DoReFa activation quantizer (clip in the
# unscaled domain, scale to [0,15], round-to-nearest-even via the 2^23 trick,
# optional rescale) in a single Vector-engine pass.
QUANT_OP = _register_dve_op(
    "QUANT_CRS_ANT",
    Spec(
        body=(minn(relu(Src0 * C0), C0) + C1 - C1) * C2,
        reference=lambda in0, in1, s0, s1, imm2: _q(
            (in0.astype(np.float32) * np.float32(s0)).astype(np.float32), s0, s1, imm2
        ),
    ),
)

# Same quantizer applied to (Src0 + Src1)*C0 -- fuses the residual add.
QUANT_RES_OP = _register_dve_op(
    "QUANT_RES_ANT",
    Spec(
        body=(minn(relu((Src0 + Src1) * C0), C0) + C1 - C1) * C2,
        reference=lambda in0, in1, s0, s1, imm2: _q(
            (
                (
                    in0.astype(np.float32).reshape(in0.shape[0], -1)
                    + in1.astype(np.float32).reshape(in1.shape[0], -1)
                ).astype(np.float32)
                * np.float32(s0)
            ).astype(np.float32),
            s0, s1, imm2,
        ).reshape(in0.shape),
    ),
)

N_CORES = 8
P = 128          # channels == partitions
NIMG = 8         # images per core
H = W = 28
HP = 30          # padded width / logical padded height
HR = 31          # allocated rows per image (junk-run overflow row)
HF = 14          # rows per chunk
NCH = NIMG * 2   # chunks per core
NRUN = HF * HP   # 420: moving-operand run per tap
MAGIC = float(2 ** 23)
F32 = mybir.dt.float32
F16 = mybir.dt.float16
BF16 = mybir.dt.bfloat16
AF = mybir.ActivationFunctionType
OP = mybir.AluOpType

CONV_GROUP = 3   # psum tiles in flight per conv group


def _quant_int(w: np.ndarray) -> np.ndarray:
    """DoReFa 4-bit weight quantization -> integer numerators m (wq = m/15)."""
    t = np.tanh(w.astype(np.float32))
    mx = np.max(np.abs(t))
    tq = t / (np.float32(2.0) * mx) + np.float32(0.5)
    j = np.round(tq * np.float32(15.0))
    return (np.float32(2.0) * j - np.float32(15.0)).astype(np.float32)


def _weights_lhsT(m: np.ndarray, dtype) -> np.ndarray:
    """[o,i,ky,kx] integer weights -> lhsT layout [i, tap, o]."""
    return np.ascontiguousarray(m.transpose(1, 2, 3, 0).reshape(P, 9, P)).astype(dtype)


def _emit(nc, tc):
    x_d = nc.dram_tensor("x", [NIMG, P, H, W], F32, kind="ExternalInput").ap()
    w1_d = nc.dram_tensor("wq1", [P, 9, P], F16, kind="ExternalInput").ap()
    w2_d = nc.dram_tensor("wq2", [P, 9, P], BF16, kind="ExternalInput").ap()
    gb_d = nc.dram_tensor("gb", [P, 4], F32, kind="ExternalInput").ap()
    eye_d = nc.dram_tensor("eye", [P, P], F32, kind="ExternalInput").ap()
    out_d = nc.dram_tensor("out", [NIMG, P, H, W], F32, kind="ExternalOutput").ap()
    wu_d = nc.dram_tensor("wu", [P], F32, kind="ExternalOutput").ap()

    rg = [list(range(N_CORES))]

    with (
        tc.tile_pool(name="persist", bufs=1) as pp,
        tc.tile_pool(name="rot", bufs=2) as rp,
        tc.tile_pool(name="fin", bufs=4) as fp,
        tc.tile_pool(name="cpsum", bufs=6, space="PSUM") as pcp,
        tc.tile_pool(name="tpsum", bufs=1, space="PSUM") as tps,
        tc.tile_pool(name="dram", bufs=1, space="DRAM") as dp,
    ):
        # ---- warmup collective: absorb ncfw first-call + core-skew cost ----
        wu_in = dp.tile([2, P], F32, tag="wuin", name="wuin")
        wu_out = dp.tile([N_CORES * 2, P], F32, tag="wuout", name="wuout")
        nc.gpsimd.dma_start(out=wu_in.opt(), in_=gb_d[:, 0:2])
        nc.gpsimd.collective_compute(
            "AllGather", OP.bypass, replica_groups=rg,
            ins=[wu_in.opt()], outs=[wu_out.opt()],
        )

        xpad = pp.tile([P, NIMG, HP, HP], F32, tag="xpad")
        xh = pp.tile([P, NIMG, HR, HP], F16, tag="xh")
        a1 = pp.tile([P, NIMG, HR, HP], BF16, tag="a1")
        raw1 = pp.tile([P, NIMG, H, W], F32, tag="raw1")
        raw2 = pp.tile([P, NIMG, H, W], F32, tag="raw2")
        w1s = pp.tile([P, 9, P], F16, tag="w1s")
        w2s = pp.tile([P, 9, P], BF16, tag="w2s")
        gbs = pp.tile([P, 4], F32, tag="gbs")
        eyes = pp.tile([P, P], F32, tag="eyes")

        # ---- zero padding borders ----
        nc.vector.memset(xpad[:, :, 0, :], 0.0)
        nc.vector.memset(xpad[:, :, HP - 1, :], 0.0)
        nc.vector.memset(xpad[:, :, 1 : HP - 1, 0], 0.0)
        nc.vector.memset(xpad[:, :, 1 : HP - 1, HP - 1], 0.0)
        for t in (xh, a1):  # row 30 is junk-run overflow: must be finite
            nc.vector.memset(t[:, :, HR - 1, :], 0.0)
        nc.vector.memset(a1[:, :, 0, :], 0.0)
        nc.vector.memset(a1[:, :, HP - 1, :], 0.0)
        nc.vector.memset(a1[:, :, 1 : HP - 1, 0], 0.0)
        nc.vector.memset(a1[:, :, 1 : HP - 1, HP - 1], 0.0)

        # ---- stream x in (two DMA queues); round to fp16 per image ----
        for n in range(NIMG):
            q = nc.sync if n % 2 == 0 else nc.scalar
            q.dma_start(out=xpad[:, n, 1 : 1 + H, 1 : 1 + W], in_=x_d[n])
            nc.vector.tensor_copy(out=xh[:, n, 0:HP, :], in_=xpad[:, n])
            if n == 1:
                nc.scalar.dma_start(out=w1s[:], in_=w1_d)
        nc.sync.dma_start(out=w2s[:], in_=w2_d)
        nc.sync.dma_start(out=gbs[:], in_=gb_d)
        nc.sync.dma_start(out=eyes[:], in_=eye_d)

        # pre-load the scalar engine's Sqrt activation table while conv1 runs:
        # the first Sqrt otherwise pays a ~1.3us ACT_TABLE_LOAD right on the
        # post-AllReduce critical path.
        warm = pp.tile([P, 1], F32, tag="warm")
        nc.vector.memset(warm[:], 1.0)
        nc.scalar.activation(out=warm[:], in_=warm[:], func=AF.Sqrt)

        def conv(pieces, wsb, rawbuf, stbuf):
            """9-tap shifted conv, contiguous 420-elem moving operands.
            PSUM tile is [P, 14, 30]; columns 28/29 are junk. ACT copies the
            real columns to SBUF; DVE bn_stats accumulates per-row stats."""
            flats = {}
            for pi, piece in enumerate(pieces):
                for n in range(NIMG):
                    flats[(pi, n)] = piece[:, n].rearrange("p h w -> p (h w)")
            nmm = 9 * len(pieces)
            # Small leading groups so the PE starts as soon as image 0's
            # input is ready, instead of waiting for images 0 AND 1.
            groups = [[0], [1, 2]] + [
                list(range(gs, min(gs + CONV_GROUP, NCH)))
                for gs in range(3, NCH, CONV_GROUP)
            ]
            for grp in groups:
                pt = {
                    ci: pcp.tile([P, HF, HP], F32, tag="cps", name=f"cps{ci}")
                    for ci in grp
                }
                for t in range(9):
                    dy, dx = divmod(t, 3)
                    for pi in range(len(pieces)):
                        k = t * len(pieces) + pi
                        for ci in grp:
                            n, hh = divmod(ci, 2)
                            off = (hh * HF + dy) * HP + dx
                            nc.tensor.matmul(
                                pt[ci][:],
                                wsb[:, t, :],
                                flats[(pi, n)][:, off : off + NRUN],
                                start=(k == 0),
                                stop=(k == nmm - 1),
                            )
                for ci in grp:
                    n, hh = divmod(ci, 2)
                    h0 = hh * HF
                    nc.scalar.activation(
                        out=rawbuf[:, n, h0 : h0 + HF, :],
                        in_=pt[ci][:, :, 0:W],
                        func=AF.Copy,
                    )
                    nc.vector.bn_stats(
                        out=stbuf[:, 6 * ci : 6 * (ci + 1)],
                        in_=rawbuf[:, n, h0 : h0 + HF, :].rearrange("p h w -> p (h w)"),
                    )

        def bn_scalars(ph, stbuf, c_mean, c_ey2, g_col, b_col, fold_scale):
            """Cross-core stat AllReduce + BN affine coefficients.

            Per-core (mean, E[y^2]) is PE-transposed to a [2,P] layout so the
            1KB each-way DMAs are 2 big packets instead of 128 tiny ones,
            AllReduce(add)'d across cores, and transposed back into PSUM.
            c_mean/c_ey2 fold the 1/(n_cores*scale) normalization into the
            scalar math; BN_EPS is dropped (vars here are O(1e2..1e5), the
            1e-5 eps is ~1e-9 relative).

            Returns (scaleA, biasB) with
              scaleA = rsqrt(var)*gamma * fold_scale   (raw -> bn domain)
              biasB  = beta - mean*rsqrt(var)*gamma
            """

            def vt(tag):
                return pp.tile([P, 1], F32, tag=f"{tag}{ph}", name=f"{tag}{ph}")

            cpk = pp.tile([P, 2], F32, tag=f"cpk{ph}", name=f"cpk{ph}")
            m2l = vt("m2l")
            nc.vector.bn_aggr(out=cpk[:], in_=stbuf[:])
            nc.vector.tensor_mul(out=m2l[:], in0=cpk[:, 0:1], in1=cpk[:, 0:1])
            nc.vector.tensor_add(out=cpk[:, 1:2], in0=cpk[:, 1:2], in1=m2l[:])
            # transpose [P,2] -> [2,P] on the (idle) PE, copy to SBUF, 1 DMA
            t2p = tps.tile([2, P], F32, tag="t2p", name=f"t2p{ph}")
            nc.tensor.transpose(t2p[:], cpk[:], eyes[:])
            c2s = pp.tile([2, P], F32, tag=f"c2s{ph}", name=f"c2s{ph}")
            nc.scalar.activation(out=c2s[:], in_=t2p[:], func=AF.Copy)
            cin = dp.tile([2, P], F32, tag=f"cin{ph}", name=f"cin{ph}")
            cout = dp.tile([2, P], F32, tag=f"cout{ph}", name=f"cout{ph}")
            nc.sync.dma_start(out=cin[:], in_=c2s[:])
            if ph == 1:
                # Hold this trigger until the warmup collective has fully
                # completed: a trigger that lands while ncfw is busy pays a
                # ~16us slow re-arm, one landing on an idle ncfw ~0.3us. The
                # wu_out-consuming store blocks the gpsimd queue (and only
                # it) until the warmup AllGather is done.
                nc.gpsimd.dma_start(out=wu_d, in_=wu_out[0, :])
            nc.gpsimd.collective_compute(
                "AllReduce", OP.add, replica_groups=rg,
                ins=[cin.opt()], outs=[cout.opt()],
            )
            c2p = pp.tile([2, P], F32, tag=f"c2p{ph}", name=f"c2p{ph}")
            nc.sync.dma_start(out=c2p[:], in_=cout[:])
            stp = tps.tile([P, 2], F32, tag="stp", name=f"stp{ph}")
            nc.tensor.transpose(stp[:], c2p[:], eyes[0:2, 0:2])

            mn, m2, u, s, r = vt("mn"), vt("m2"), vt("u"), vt("s"), vt("r")
            # mean = c_mean * sum; mean^2 on DVE (no ACT Square table switch)
            nc.scalar.activation(out=mn[:], in_=stp[:, 0:1], func=AF.Copy, scale=c_mean)
            nc.vector.tensor_mul(out=m2[:], in0=mn[:], in1=mn[:])
            # u = var = E[y^2] - mean^2
            nc.vector.scalar_tensor_tensor(
                out=u[:], in0=stp[:, 1:2], scalar=c_ey2, in1=m2[:],
                op0=OP.mult, op1=OP.subtract,
            )
            nc.scalar.activation(out=s[:], in_=u[:], func=AF.Sqrt)
            nc.vector.reciprocal(out=r[:], in_=s[:])
            t0, t1, jk = vt("t0"), vt("t1"), vt("jk")
            # one Newton step r <- r*(1.5 - 0.5*u*r^2): recip(sqrt()) is
            # accurate to ~1e-4; one quadratic step lands ~1e-8 relative.
            nc.vector.tensor_mul(out=t0[:], in0=r[:], in1=r[:])
            nc.vector.tensor_mul(out=t1[:], in0=t0[:], in1=u[:])
            nc.vector.affine_mul_reduce(
                out=r[:], accum_out=jk[:], in0=t1[:], in1=r[:],
                scale=-0.5, bias=1.5,
            )
            rgm, scaleA, b0, biasB = vt("rg"), vt("sA"), vt("b0"), vt("bB")
            nc.vector.tensor_mul(out=rgm[:], in0=r[:], in1=gbs[:, g_col : g_col + 1])
            if fold_scale == 1.0:
                scaleA = rgm
            else:
                nc.vector.tensor_scalar(
                    out=scaleA[:], in0=rgm[:], scalar1=fold_scale, scalar2=None, op0=OP.mult
                )
            nc.vector.tensor_mul(out=b0[:], in0=mn[:], in1=rgm[:])
            nc.vector.tensor_sub(
                out=biasB[:], in0=gbs[:, b_col : b_col + 1], in1=b0[:]
            )
            return scaleA, biasB

        # ================= phase 1: conv1 + BN1 stats =================
        st1 = pp.tile([P, NCH * 6], F32, tag="st1")
        conv([xh], w1s, raw1, st1)
        sA1, bB1 = bn_scalars(
            1, st1, 1.0 / (N_CORES * 15.0), 1.0 / (N_CORES * 225.0), 0, 1, 1.0 / 15.0
        )

        # PE/HAM warm-up: 16 junk matmuls queued right behind the AR1 return
        # transpose. They execute during the ~5us post-collective scalar
        # chain (the PE sat idle ~25us during the AllReduce wait, so HAM has
        # re-throttled it to 1.2 GHz; ~3.4us of sustained activity restores
        # 2.4 GHz before conv2's first real matmul).
        jm = pcp.tile([P, HF, HP], F32, tag="cps", name="jwarm")
        xh0 = xh[:, 0].rearrange("p h w -> p (h w)")
        for _ in range(16):
            nc.tensor.matmul(jm[:], w1s[:, 0, :], xh0[:, 0:NRUN], start=True, stop=True)

        # ============ act1 quantization -> integers in bf16 (per image) ============
        # image 0 is processed in two row-bands so conv2's first chunk (which
        # needs only padded rows 0..15) can start before the whole image is
        # quantized -- this sits on the serial post-collective path.
        for n in range(NIMG):
            bands = ((0, 16), (16, H)) if n <= 1 else ((0, H),)
            for r0, r1 in bands:
                u = rp.tile([P, r1 - r0, W], F32, tag="uq", name=f"uq{n}_{r0}")
                nc.scalar.activation(
                    out=u[:], in_=raw1[:, n, r0:r1, :], func=AF.Relu,
                    bias=bB1[:], scale=sA1[:],
                )
                nc.vector._custom_dve(
                    QUANT_OP,
                    out=a1[:, n, 1 + r0 : 1 + r1, 1 : 1 + W],
                    in0=u[:],
                    s0=15.0,
                    s1=MAGIC,
                    imm2=1.0,
                )

        # ================= phase 2: conv2 + BN2 stats =================
        st2 = pp.tile([P, NCH * 6], F32, tag="st2")
        conv([a1], w2s, raw2, st2)
        sA2, bB2 = bn_scalars(
            2, st2, 1.0 / (N_CORES * 225.0), 1.0 / (N_CORES * 225.0 * 225.0), 2, 3, 1.0 / 225.0
        )

        # ========== final: bn2 + residual + qact ==========
        # ACT applies the BN affine two images at a time (halves the per-op
        # overhead; the per-image DVE quant becomes the pacer), one fused DVE
        # op per image does residual add + clip + round + rescale; output
        # DMAs rotate over three queues, and the last image's store is split
        # across two queues to shorten the tail.
        for n in range(NIMG):
            p1 = fp.tile([P, H * W], F32, tag="p1", name=f"p1_{n}")
            nc.scalar.activation(
                out=p1[:],
                in_=raw2[:, n].rearrange("p h w -> p (h w)"),
                func=AF.Identity,
                bias=bB2[:],
                scale=sA2[:],
            )
            og = fp.tile([P, H * W], F32, tag="og", name=f"og_{n}")
            nc.vector._custom_dve(
                QUANT_RES_OP,
                out=og[:],
                in0=xpad[:, n, 1 : 1 + H, 1 : 1 + W],
                in1=p1[:],
                s0=15.0,
                s1=MAGIC,
                imm2=1.0 / 15.0,
            )
            if n == NIMG - 1:
                half = (H // 2) * W
                nc.sync.dma_start(out=out_d[n, :, 0 : H // 2, :], in_=og[:, 0:half])
                nc.scalar.dma_start(out=out_d[n, :, H // 2 :, :], in_=og[:, half:])
            else:
                q = (nc.sync, nc.gpsimd, nc.scalar)[n % 3]
                q.dma_start(out=out_d[n], in_=og[:])


_PROGRAM = None


def get_program():
    global _PROGRAM
    if _PROGRAM is None:
        nc = bacc.Bacc(
            "TRN2",
            target_bir_lowering=False,
            debug=False,
            enable_asserts=True,
            num_devices=N_CORES,
        )
        with tile.TileContext(nc, num_cores=N_CORES) as tc:
            _emit(nc, tc)
        nc.compile()
        _PROGRAM = nc
    return _PROGRAM


def make_in_maps(inputs):
    x = np.asarray(inputs["x"], np.float32)
    m1 = _quant_int(np.asarray(inputs["w1"], np.float32))
    mask = (np.asarray(inputs["mask2"], np.float32) > 0.5).astype(np.float32)
    m2 = _quant_int(np.asarray(inputs["w2"], np.float32) * mask)
    wq1 = _weights_lhsT(m1, np.float16)
    wq2 = _weights_lhsT(m2, ml_dtypes.bfloat16)
    gb = np.stack(
        [
            np.asarray(inputs["gamma1"], np.float32),
            np.asarray(inputs["beta1"], np.float32),
            np.asarray(inputs["gamma2"], np.float32),
            np.asarray(inputs["beta2"], np.float32),
        ],
        axis=1,
    )
    gb = np.ascontiguousarray(gb)
    eye = np.eye(P, dtype=np.float32)
    return [
        {
            "x": np.ascontiguousarray(x[NIMG * i : NIMG * (i + 1)]),
            "wq1": wq1,
            "wq2": wq2,
            "gb": gb,
            "eye": eye,
        }
        for i in range(N_CORES)
    ]


def run(inputs, **kwargs) -> bass_utils.BassKernelResults:
    nc = get_program()
    return bass_utils.run_bass_kernel_spmd(
        nc, make_in_maps(inputs), core_ids=list(range(N_CORES)), **kwargs
    )


def kernel(**inputs) -> np.ndarray:
    res = run(inputs)
    return np.concatenate(
        [res.results[i]["out"] for i in range(N_CORES)], axis=0
    ).astype(np.float32)


# revision 26
# speedup vs baseline: 1.2163x; 1.2163x over previous
"""Trainium2 Bass kernel for a DoReFa-quantized ResNet BasicBlock.

    out = qact(bn2(conv3x3(qact(bn1(conv3x3(x, qw(w1)))), qw(w2*mask))) + x)

Full inputs: x (64,128,28,28) f32, w1/w2/mask2 (128,128,3,3), BN params (128,).
Data-parallel over 8 NeuronCores (8 images each); BN batch statistics are
exchanged with two tiny AllReduce collectives.

Numerical scheme (validated against the jax reference, ~1.1e-2 rel-L2 vs the
2e-2 gate; inputs are deterministic):
 - DoReFa weights quantize onto the grid m/15, m an odd integer in [-15,15].
   The integers m are computed on host (cheap, and bit-matches jax's rounding)
   and shipped as integer-valued fp16/bf16 tensors (exactly representable).
 - conv1 runs on x rounded to fp16 (single pass). The products int4 x fp16
   are exact in fp32 PSUM; the only error is the fp16 rounding of x. A bf16
   hi+lo two-pass conv1 (1.5e-3) was measured SLOWER end-to-end: all cores
   must issue the BN1-stats collective trigger before the ncfw warmup chain
   completes (~90-105us wall incl. launch skew), which the ~62us hi/lo conv1
   misses but the ~32us fp16 conv1 makes comfortably.
 - conv2's input activations are quantized to j/15, j in 0..15. Feeding the
   integers j as bf16 makes conv2 an exact integer matmul (products <= 225,
   sums <= 1152*225 < 2^24: exact in fp32 PSUM) at full bf16 PE throughput.
 - 3x3 conv = 9 shifted [128 x 128] matmuls accumulated in PSUM over a
   zero-padded [C=128 part, img, 31, 30] SBUF image layout. Each tap's moving
   operand is a fully CONTIGUOUS 420-element run (14 rows x 30 incl. 2 junk
   columns/row); junk columns land in unused PSUM columns.
 - the activation quantizer (clip / x15 / round-to-nearest-even via the
   (t + 2^23) - 2^23 trick / rescale) is one fused custom Vector-engine op;
   the residual variant also folds in the skip-connection add.
 - BN statistics: DVE bn_stats/bn_aggr per chunk -> per-core (mean, E[y^2])
   in a [P,2] tile -> PE-transpose to [2,P] (a [P,2]-shaped DMA moves 8 bytes
   per partition = 128 packets = ~4us packet-rate-bound; the transposed
   [2,P] layout is 2 big packets, ~1us) -> 1KB AllReduce(add) -> DMA back to
   a 2-partition tile -> PE-transpose back into PSUM -> rsqrt via ACT sqrt +
   DVE reciprocal + 1 Newton step (ACT's Rsqrt is blocked for accuracy). A
   junk ACT Sqrt during startup pre-loads the Sqrt table so the ~1.3us
   ACT_TABLE_LOAD stays off the post-collective critical path. The trigger
   DMAs ride the sync queue: the gpsimd queue sits blocked behind the warmup
   collective and was measured to delay the stats AllReduce by ~15us.
 - a throwaway AllGather issued at kernel start absorbs the ncfw
   first-collective setup cost (~75us) in parallel with input DMA + conv1;
   its output store is deferred to the end of the program so no engine queue
   blocks on the collective's completion.
"""

import os
import sys

import numpy as np

for _p in ("/opt/trn_rl_repo",):
    if _p not in sys.path and os.path.isdir(_p):
        sys.path.insert(0, _p)

import ml_dtypes  # noqa: E402

from concourse import bacc, mybir, tile  # noqa: E402
from concourse import bass_utils  # noqa: E402
from concourse import dve_ops  # noqa: E402
from concourse.dve_spec import C0, C1, C2, Spec, Src0, Src1, lower, minn, relu  # noqa: E402
from concourse.dve_spec import _has_src1 as has_src1  # noqa: E402
from concourse.dve_uop import DveOpSpec  # noqa: E402


def _register_dve_op(name, spec):
    for op in dve_ops.OPS:
        if op.name == name:
            return op
    row = dve_ops._CUSTOM_DVE_ROW_BASE + len(dve_ops.OPS)
    assert row < 0x20
    shas = {}
    for ver in ("v3", "v4"):
        shas[ver] = DveOpSpec(
            name=name, opcode=row, uops=lower(spec, ver=ver), rd1_en=has_src1(spec)
        ).sha(ver)
    op = dve_ops.DveOp(name, spec, subdim=False, uops_sha=shas)
    dve_ops.OPS.append(op)
    dve_ops.CUSTOM_DVE_SPECS[name] = spec
    dve_ops._SUB_OPCODE_FOR_NAME[name] = row
    return op


def _q(t, s0, s1, imm2):
    f = np.float32
    t = np.minimum(np.maximum(t, f(0.0)), f(s0)).astype(np.float32)
    t = (t + f(s1)).astype(np.float32)
    t = (t - f(s1)).astype(np.float32)
    return (t * f(imm2)).astype(np.float32)


# out = (min(relu(in*C0), C0) + C1 - C1) * C2 : with C0=15, C1=2^23,
# C2 in {1, 1/15} this is the whole DoReFa activation quantizer (clip in the
# unscaled domain, scale to [0,15], round-to-nearest-even via the 2^23 trick,
# optional rescale) in a single Vector-engine pass.
QUANT_OP = _register_dve_op(
    "QUANT_CRS_ANT",
    Spec(
        body=(minn(relu(Src0 * C0), C0) + C1 - C1) * C2,
        reference=lambda in0, in1, s0, s1, imm2: _q(
            (in0.astype(np.float32) * np.float32(s0)).astype(np.float32), s0, s1, imm2
        ),
    ),
)

# Same quantizer applied to (Src0 + Src1)*C0 -- fuses the residual add.
QUANT_RES_OP = _register_dve_op(
    "QUANT_RES_ANT",
    Spec(
        body=(minn(relu((Src0 + Src1) * C0), C0) + C1 - C1) * C2,
        reference=lambda in0, in1, s0, s1, imm2: _q(
            (
                (
                    in0.astype(np.float32).reshape(in0.shape[0], -1)
                    + in1.astype(np.float32).reshape(in1.shape[0], -1)
                ).astype(np.float32)
                * np.float32(s0)
            ).astype(np.float32),
            s0, s1, imm2,
        ).reshape(in0.shape),
    ),
)

N_CORES = 8
P = 128          # channels == partitions
NIMG = 8         # images per core
H = W = 28
HP = 30          # padded width / logical padded height
HR = 31          # allocated rows per image (junk-run overflow row)
HF = 14          # rows per chunk
NCH = NIMG * 2   # chunks per core
NRUN = HF * HP   # 420: moving-operand run per tap
MAGIC = float(2 ** 23)
F32 = mybir.dt.float32
F16 = mybir.dt.float16
BF16 = mybir.dt.bfloat16
AF = mybir.ActivationFunctionType
OP = mybir.AluOpType

CONV_GROUP = 3   # psum tiles in flight per conv group


def _quant_int(w: np.ndarray) -> np.ndarray:
    """DoReFa 4-bit weight quantization -> integer numerators m (wq = m/15)."""
    t = np.tanh(w.astype(np.float32))
    mx = np.max(np.abs(t))
    tq = t / (np.float32(2.0) * mx) + np.float32(0.5)
    j = np.round(tq * np.float32(15.0))
    return (np.float32(2.0) * j - np.float32(15.0)).astype(np.float32)


def _weights_lhsT(m: np.ndarray, dtype) -> np.ndarray:
    """[o,i,ky,kx] integer weights -> lhsT layout [i, tap, o]."""
    return np.ascontiguousarray(m.transpose(1, 2, 3, 0).reshape(P, 9, P)).astype(dtype)


def _emit(nc, tc):
    x_d = nc.dram_tensor("x", [NIMG, P, H, W], F32, kind="ExternalInput").ap()
    w1_d = nc.dram_tensor("wq1", [P, 9, P], F16, kind="ExternalInput").ap()
    w2_d = nc.dram_tensor("wq2", [P, 9, P], BF16, kind="ExternalInput").ap()
    gb_d = nc.dram_tensor("gb", [P, 4], F32, kind="ExternalInput").ap()
    eye_d = nc.dram_tensor("eye", [P, P], F32, kind="ExternalInput").ap()
    out_d = nc.dram_tensor("out", [NIMG, P, H, W], F32, kind="ExternalOutput").ap()
    wu_d = nc.dram_tensor("wu", [P], F32, kind="ExternalOutput").ap()

    rg = [list(range(N_CORES))]

    with (
        tc.tile_pool(name="persist", bufs=1) as pp,
        tc.tile_pool(name="rot", bufs=2) as rp,
        tc.tile_pool(name="fin", bufs=4) as fp,
        tc.tile_pool(name="cpsum", bufs=6, space="PSUM") as pcp,
        tc.tile_pool(name="tpsum", bufs=1, space="PSUM") as tps,
        tc.tile_pool(name="dram", bufs=1, space="DRAM") as dp,
    ):
        # ---- warmup collective: absorb ncfw first-call + core-skew cost ----
        wu_in = dp.tile([2, P], F32, tag="wuin", name="wuin")
        wu_out = dp.tile([N_CORES * 2, P], F32, tag="wuout", name="wuout")
        nc.gpsimd.dma_start(out=wu_in.opt(), in_=gb_d[:, 0:2])
        nc.gpsimd.collective_compute(
            "AllGather", OP.bypass, replica_groups=rg,
            ins=[wu_in.opt()], outs=[wu_out.opt()],
        )

        xpad = pp.tile([P, NIMG, HP, HP], F32, tag="xpad")
        xh = pp.tile([P, NIMG, HR, HP], F16, tag="xh")
        a1 = pp.tile([P, NIMG, HR, HP], BF16, tag="a1")
        raw1 = pp.tile([P, NIMG, H, W], F32, tag="raw1")
        raw2 = pp.tile([P, NIMG, H, W], F32, tag="raw2")
        w1s = pp.tile([P, 9, P], F16, tag="w1s")
        w2s = pp.tile([P, 9, P], BF16, tag="w2s")
        gbs = pp.tile([P, 4], F32, tag="gbs")
        eyes = pp.tile([P, P], F32, tag="eyes")

        # ---- zero padding borders ----
        nc.vector.memset(xpad[:, :, 0, :], 0.0)
        nc.vector.memset(xpad[:, :, HP - 1, :], 0.0)
        nc.vector.memset(xpad[:, :, 1 : HP - 1, 0], 0.0)
        nc.vector.memset(xpad[:, :, 1 : HP - 1, HP - 1], 0.0)
        for t in (xh, a1):  # row 30 is junk-run overflow: must be finite
            nc.vector.memset(t[:, :, HR - 1, :], 0.0)
        nc.vector.memset(a1[:, :, 0, :], 0.0)
        nc.vector.memset(a1[:, :, HP - 1, :], 0.0)
        nc.vector.memset(a1[:, :, 1 : HP - 1, 0], 0.0)
        nc.vector.memset(a1[:, :, 1 : HP - 1, HP - 1], 0.0)

        # ---- stream x in (two DMA queues); round to fp16 per image ----
        for n in range(NIMG):
            q = nc.sync if n % 2 == 0 else nc.scalar
            q.dma_start(out=xpad[:, n, 1 : 1 + H, 1 : 1 + W], in_=x_d[n])
            nc.vector.tensor_copy(out=xh[:, n, 0:HP, :], in_=xpad[:, n])
            if n == 1:
                nc.scalar.dma_start(out=w1s[:], in_=w1_d)
        nc.sync.dma_start(out=w2s[:], in_=w2_d)
        nc.sync.dma_start(out=gbs[:], in_=gb_d)
        nc.sync.dma_start(out=eyes[:], in_=eye_d)

        # pre-load the scalar engine's Sqrt activation table while conv1 runs:
        # the first Sqrt otherwise pays a ~1.3us ACT_TABLE_LOAD right on the
        # post-AllReduce critical path.
        warm = pp.tile([P, 1], F32, tag="warm")
        nc.vector.memset(warm[:], 1.0)
        nc.scalar.activation(out=warm[:], in_=warm[:], func=AF.Sqrt)

        def conv(pieces, wsb, rawbuf, stbuf):
            """9-tap shifted conv, contiguous 420-elem moving operands.
            PSUM tile is [P, 14, 30]; columns 28/29 are junk. ACT copies the
            real columns to SBUF; DVE bn_stats accumulates per-row stats."""
            flats = {}
            for pi, piece in enumerate(pieces):
                for n in range(NIMG):
                    flats[(pi, n)] = piece[:, n].rearrange("p h w -> p (h w)")
            nmm = 9 * len(pieces)
            # Small leading groups so the PE starts as soon as image 0's
            # input is ready, instead of waiting for images 0 AND 1.
            groups = [[0], [1, 2]] + [
                list(range(gs, min(gs + CONV_GROUP, NCH)))
                for gs in range(3, NCH, CONV_GROUP)
            ]
            for grp in groups:
                pt = {
                    ci: pcp.tile([P, HF, HP], F32, tag="cps", name=f"cps{ci}")
                    for ci in grp
                }
                for t in range(9):
                    dy, dx = divmod(t, 3)
                    for pi in range(len(pieces)):
                        k = t * len(pieces) + pi
                        for ci in grp:
                            n, hh = divmod(ci, 2)
                            off = (hh * HF + dy) * HP + dx
                            nc.tensor.matmul(
                                pt[ci][:],
                                wsb[:, t, :],
                                flats[(pi, n)][:, off : off + NRUN],
                                start=(k == 0),
                                stop=(k == nmm - 1),
                            )
                for ci in grp:
                    n, hh = divmod(ci, 2)
                    h0 = hh * HF
                    nc.scalar.activation(
                        out=rawbuf[:, n, h0 : h0 + HF, :],
                        in_=pt[ci][:, :, 0:W],
                        func=AF.Copy,
                    )
                    nc.vector.bn_stats(
                        out=stbuf[:, 6 * ci : 6 * (ci + 1)],
                        in_=rawbuf[:, n, h0 : h0 + HF, :].rearrange("p h w -> p (h w)"),
                    )

        def bn_scalars(ph, stbuf, c_mean, c_ey2, g_col, b_col, fold_scale):
            """Cross-core stat AllReduce + BN affine coefficients.

            Per-core (mean, E[y^2]) is PE-transposed to a [2,P] layout so the
            1KB each-way DMAs are 2 big packets instead of 128 tiny ones,
            AllReduce(add)'d across cores, and transposed back into PSUM.
            c_mean/c_ey2 fold the 1/(n_cores*scale) normalization into the
            scalar math; BN_EPS is dropped (vars here are O(1e2..1e5), the
            1e-5 eps is ~1e-9 relative).

            Returns (scaleA, biasB) with
              scaleA = rsqrt(var)*gamma * fold_scale   (raw -> bn domain)
              biasB  = beta - mean*rsqrt(var)*gamma
            """

            def vt(tag):
                return pp.tile([P, 1], F32, tag=f"{tag}{ph}", name=f"{tag}{ph}")

            cpk = pp.tile([P, 2], F32, tag=f"cpk{ph}", name=f"cpk{ph}")
            m2l = vt("m2l")
            nc.vector.bn_aggr(out=cpk[:], in_=stbuf[:])
            nc.vector.tensor_mul(out=m2l[:], in0=cpk[:, 0:1], in1=cpk[:, 0:1])
            nc.vector.tensor_add(out=cpk[:, 1:2], in0=cpk[:, 1:2], in1=m2l[:])
            # transpose [P,2] -> [2,P] on the (idle) PE, copy to SBUF, 1 DMA
            t2p = tps.tile([2, P], F32, tag="t2p", name=f"t2p{ph}")
            nc.tensor.transpose(t2p[:], cpk[:], eyes[:])
            c2s = pp.tile([2, P], F32, tag=f"c2s{ph}", name=f"c2s{ph}")
            nc.scalar.activation(out=c2s[:], in_=t2p[:], func=AF.Copy)
            cin = dp.tile([2, P], F32, tag=f"cin{ph}", name=f"cin{ph}")
            cout = dp.tile([2, P], F32, tag=f"cout{ph}", name=f"cout{ph}")
            nc.sync.dma_start(out=cin[:], in_=c2s[:])
            if ph == 1:
                # Hold this trigger until the warmup collective has fully
                # completed: a trigger that lands while ncfw is busy pays a
                # ~16us slow re-arm, one landing on an idle ncfw ~0.3us. The
                # wu_out-consuming store blocks the gpsimd queue (and only
                # it) until the warmup AllGather is done.
                nc.gpsimd.dma_start(out=wu_d, in_=wu_out[0, :])
            nc.gpsimd.collective_compute(
                "AllReduce", OP.add, replica_groups=rg,
                ins=[cin.opt()], outs=[cout.opt()],
            )
            c2p = pp.tile([2, P], F32, tag=f"c2p{ph}", name=f"c2p{ph}")
            nc.sync.dma_start(out=c2p[:], in_=cout[:])
            stp = tps.tile([P, 2], F32, tag="stp", name=f"stp{ph}")
            nc.tensor.transpose(stp[:], c2p[:], eyes[0:2, 0:2])

            mn, m2, u, s, r = vt("mn"), vt("m2"), vt("u"), vt("s"), vt("r")
            # mean = c_mean * sum; mean^2 on DVE (no ACT Square table switch)
            nc.scalar.activation(out=mn[:], in_=stp[:, 0:1], func=AF.Copy, scale=c_mean)
            nc.vector.tensor_mul(out=m2[:], in0=mn[:], in1=mn[:])
            # u = var = E[y^2] - mean^2
            nc.vector.scalar_tensor_tensor(
                out=u[:], in0=stp[:, 1:2], scalar=c_ey2, in1=m2[:],
                op0=OP.mult, op1=OP.subtract,
            )
            nc.scalar.activation(out=s[:], in_=u[:], func=AF.Sqrt)
            nc.vector.reciprocal(out=r[:], in_=s[:])
            t0, t1, jk = vt("t0"), vt("t1"), vt("jk")
            # one Newton step r <- r*(1.5 - 0.5*u*r^2): recip(sqrt()) is
            # accurate to ~1e-4; one quadratic step lands ~1e-8 relative.
            nc.vector.tensor_mul(out=t0[:], in0=r[:], in1=r[:])
            nc.vector.tensor_mul(out=t1[:], in0=t0[:], in1=u[:])
            nc.vector.affine_mul_reduce(
                out=r[:], accum_out=jk[:], in0=t1[:], in1=r[:],
                scale=-0.5, bias=1.5,
            )
            rgm, scaleA, b0, biasB = vt("rg"), vt("sA"), vt("b0"), vt("bB")
            nc.vector.tensor_mul(out=rgm[:], in0=r[:], in1=gbs[:, g_col : g_col + 1])
            if fold_scale == 1.0:
                scaleA = rgm
            else:
                nc.vector.tensor_scalar(
                    out=scaleA[:], in0=rgm[:], scalar1=fold_scale, scalar2=None, op0=OP.mult
                )
            nc.vector.tensor_mul(out=b0[:], in0=mn[:], in1=rgm[:])
            nc.vector.tensor_sub(
                out=biasB[:], in0=gbs[:, b_col : b_col + 1], in1=b0[:]
            )
            return scaleA, biasB

        # ================= phase 1: conv1 + BN1 stats =================
        st1 = pp.tile([P, NCH * 6], F32, tag="st1")
        conv([xh], w1s, raw1, st1)
        sA1, bB1 = bn_scalars(
            1, st1, 1.0 / (N_CORES * 15.0), 1.0 / (N_CORES * 225.0), 0, 1, 1.0 / 15.0
        )

        # ============ act1 quantization -> integers in bf16 (per image) ============
        # image 0 is processed in two row-bands so conv2's first chunk (which
        # needs only padded rows 0..15) can start before the whole image is
        # quantized -- this sits on the serial post-collective path.
        for n in range(NIMG):
            bands = ((0, 16), (16, H)) if n <= 1 else ((0, H),)
            for r0, r1 in bands:
                u = rp.tile([P, r1 - r0, W], F32, tag="uq", name=f"uq{n}_{r0}")
                nc.scalar.activation(
                    out=u[:], in_=raw1[:, n, r0:r1, :], func=AF.Relu,
                    bias=bB1[:], scale=sA1[:],
                )
                nc.vector._custom_dve(
                    QUANT_OP,
                    out=a1[:, n, 1 + r0 : 1 + r1, 1 : 1 + W],
                    in0=u[:],
                    s0=15.0,
                    s1=MAGIC,
                    imm2=1.0,
                )

        # ================= phase 2: conv2 + BN2 stats =================
        st2 = pp.tile([P, NCH * 6], F32, tag="st2")
        conv([a1], w2s, raw2, st2)
        sA2, bB2 = bn_scalars(
            2, st2, 1.0 / (N_CORES * 225.0), 1.0 / (N_CORES * 225.0 * 225.0), 2, 3, 1.0 / 225.0
        )

        # ========== final: bn2 + residual + qact ==========
        # ACT applies the BN affine two images at a time (halves the per-op
        # overhead; the per-image DVE quant becomes the pacer), one fused DVE
        # op per image does residual add + clip + round + rescale; output
        # DMAs rotate over three queues, and the last image's store is split
        # across two queues to shorten the tail.
        for b0, b1 in ((0, 1), (1, 3), (3, 5), (5, 7), (7, 8)):
            nb = b1 - b0
            p1 = fp.tile([P, nb * H * W], F32, tag="p1", name=f"p1_{b0}")
            nc.scalar.activation(
                out=p1[:],
                in_=raw2[:, b0:b1].rearrange("p n h w -> p (n h w)"),
                func=AF.Identity,
                bias=bB2[:],
                scale=sA2[:],
            )
            for k in range(nb):
                n = b0 + k
                og = fp.tile([P, H * W], F32, tag="og", name=f"og_{n}")
                nc.vector._custom_dve(
                    QUANT_RES_OP,
                    out=og[:],
                    in0=xpad[:, n, 1 : 1 + H, 1 : 1 + W],
                    in1=p1[:, k * H * W : (k + 1) * H * W],
                    s0=15.0,
                    s1=MAGIC,
                    imm2=1.0 / 15.0,
                )
                if n == NIMG - 1:
                    half = (H // 2) * W
                    nc.sync.dma_start(out=out_d[n, :, 0 : H // 2, :], in_=og[:, 0:half])
                    nc.scalar.dma_start(out=out_d[n, :, H // 2 :, :], in_=og[:, half:])
                else:
                    q = (nc.sync, nc.gpsimd, nc.scalar)[n % 3]
                    q.dma_start(out=out_d[n], in_=og[:])


_PROGRAM = None


def get_program():
    global _PROGRAM
    if _PROGRAM is None:
        nc = bacc.Bacc(
            "TRN2",
            target_bir_lowering=False,
            debug=False,
            enable_asserts=True,
            num_devices=N_CORES,
        )
        with tile.TileContext(nc, num_cores=N_CORES) as tc:
            _emit(nc, tc)
        nc.compile()
        _PROGRAM = nc
    return _PROGRAM


def make_in_maps(inputs):
    x = np.asarray(inputs["x"], np.float32)
    m1 = _quant_int(np.asarray(inputs["w1"], np.float32))
    mask = (np.asarray(inputs["mask2"], np.float32) > 0.5).astype(np.float32)
    m2 = _quant_int(np.asarray(inputs["w2"], np.float32) * mask)
    wq1 = _weights_lhsT(m1, np.float16)
    wq2 = _weights_lhsT(m2, ml_dtypes.bfloat16)
    gb = np.stack(
        [
            np.asarray(inputs["gamma1"], np.float32),
            np.asarray(inputs["beta1"], np.float32),
            np.asarray(inputs["gamma2"], np.float32),
            np.asarray(inputs["beta2"], np.float32),
        ],
        axis=1,
    )
    gb = np.ascontiguousarray(gb)
    eye = np.eye(P, dtype=np.float32)
    return [
        {
            "x": np.ascontiguousarray(x[NIMG * i : NIMG * (i + 1)]),
            "wq1": wq1,
            "wq2": wq2,
            "gb": gb,
            "eye": eye,
        }
        for i in range(N_CORES)
    ]


def run(inputs, **kwargs) -> bass_utils.BassKernelResults:
    nc = get_program()
    return bass_utils.run_bass_kernel_spmd(
        nc, make_in_maps(inputs), core_ids=list(range(N_CORES)), **kwargs
    )


def kernel(**inputs) -> np.ndarray:
    res = run(inputs)
    return np.concatenate(
        [res.results[i]["out"] for i in range(N_CORES)], axis=0
    ).astype(np.float32)


# revision 28
# speedup vs baseline: 1.2213x; 1.0042x over previous
"""Trainium2 Bass kernel for a DoReFa-quantized ResNet BasicBlock.

    out = qact(bn2(conv3x3(qact(bn1(conv3x3(x, qw(w1)))), qw(w2*mask))) + x)

Full inputs: x (64,128,28,28) f32, w1/w2/mask2 (128,128,3,3), BN params (128,).
Data-parallel over 8 NeuronCores (8 images each); BN batch statistics are
exchanged with two tiny AllReduce collectives.

Numerical scheme (validated against the jax reference, ~1.1e-2 rel-L2 vs the
2e-2 gate; inputs are deterministic):
 - DoReFa weights quantize onto the grid m/15, m an odd integer in [-15,15].
   The integers m are computed on host (cheap, and bit-matches jax's rounding)
   and shipped as integer-valued fp16/bf16 tensors (exactly representable).
 - conv1 runs on x rounded to fp16 (single pass). The products int4 x fp16
   are exact in fp32 PSUM; the only error is the fp16 rounding of x. A bf16
   hi+lo two-pass conv1 (1.5e-3) was measured SLOWER end-to-end: all cores
   must issue the BN1-stats collective trigger before the ncfw warmup chain
   completes (~90-105us wall incl. launch skew), which the ~62us hi/lo conv1
   misses but the ~32us fp16 conv1 makes comfortably.
 - conv2's input activations are quantized to j/15, j in 0..15. Feeding the
   integers j as bf16 makes conv2 an exact integer matmul (products <= 225,
   sums <= 1152*225 < 2^24: exact in fp32 PSUM) at full bf16 PE throughput.
 - 3x3 conv = 9 shifted [128 x 128] matmuls accumulated in PSUM over a
   zero-padded [C=128 part, img, 31, 30] SBUF image layout. Each tap's moving
   operand is a fully CONTIGUOUS 420-element run (14 rows x 30 incl. 2 junk
   columns/row); junk columns land in unused PSUM columns.
 - the activation quantizer (clip / x15 / round-to-nearest-even via the
   (t + 2^23) - 2^23 trick / rescale) is one fused custom Vector-engine op;
   the residual variant also folds in the skip-connection add.
 - BN statistics: DVE bn_stats/bn_aggr per chunk -> per-core (mean, E[y^2])
   in a [P,2] tile -> PE-transpose to [2,P] (a [P,2]-shaped DMA moves 8 bytes
   per partition = 128 packets = ~4us packet-rate-bound; the transposed
   [2,P] layout is 2 big packets, ~1us) -> 1KB AllReduce(add) -> DMA back to
   a 2-partition tile -> PE-transpose back into PSUM -> rsqrt via ACT sqrt +
   DVE reciprocal + 1 Newton step (ACT's Rsqrt is blocked for accuracy). A
   junk ACT Sqrt during startup pre-loads the Sqrt table so the ~1.3us
   ACT_TABLE_LOAD stays off the post-collective critical path. The trigger
   DMAs ride the sync queue: the gpsimd queue sits blocked behind the warmup
   collective and was measured to delay the stats AllReduce by ~15us.
 - a throwaway AllGather issued at kernel start absorbs the ncfw
   first-collective setup cost (~75us) in parallel with input DMA + conv1;
   its output store is deferred to the end of the program so no engine queue
   blocks on the collective's completion.
"""

import os
import sys

import numpy as np

for _p in ("/opt/trn_rl_repo",):
    if _p not in sys.path and os.path.isdir(_p):
        sys.path.insert(0, _p)

import ml_dtypes  # noqa: E402

from concourse import bacc, mybir, tile  # noqa: E402
from concourse import bass_utils  # noqa: E402
from concourse import dve_ops  # noqa: E402
from concourse.dve_spec import C0, C1, C2, Spec, Src0, Src1, lower, minn, relu  # noqa: E402
from concourse.dve_spec import _has_src1 as has_src1  # noqa: E402
from concourse.dve_uop import DveOpSpec  # noqa: E402


def _register_dve_op(name, spec):
    for op in dve_ops.OPS:
        if op.name == name:
            return op
    row = dve_ops._CUSTOM_DVE_ROW_BASE + len(dve_ops.OPS)
    assert row < 0x20
    shas = {}
    for ver in ("v3", "v4"):
        shas[ver] = DveOpSpec(
            name=name, opcode=row, uops=lower(spec, ver=ver), rd1_en=has_src1(spec)
        ).sha(ver)
    op = dve_ops.DveOp(name, spec, subdim=False, uops_sha=shas)
    dve_ops.OPS.append(op)
    dve_ops.CUSTOM_DVE_SPECS[name] = spec
    dve_ops._SUB_OPCODE_FOR_NAME[name] = row
    return op


def _q(t, s0, s1, imm2):
    f = np.float32
    t = np.minimum(np.maximum(t, f(0.0)), f(s0)).astype(np.float32)
    t = (t + f(s1)).astype(np.float32)
    t = (t - f(s1)).astype(np.float32)
    return (t * f(imm2)).astype(np.float32)


# out = (min(relu(in*C0), C0) + C1 - C1) * C2 : with C0=15, C1=2^23,
# C2 in {1, 1/15} this is the whole DoReFa activation quantizer (clip in the
# unscaled domain, scale to [0,15], round-to-nearest-even via the 2^23 trick,
# optional rescale) in a single Vector-engine pass.
QUANT_OP = _register_dve_op(
    "QUANT_CRS_ANT",
    Spec(
        body=(minn(relu(Src0 * C0), C0) + C1 - C1) * C2,
        reference=lambda in0, in1, s0, s1, imm2: _q(
            (in0.astype(np.float32) * np.float32(s0)).astype(np.float32), s0, s1, imm2
        ),
    ),
)

# Same quantizer applied to (Src0 + Src1)*C0 -- fuses the residual add.
QUANT_RES_OP = _register_dve_op(
    "QUANT_RES_ANT",
    Spec(
        body=(minn(relu((Src0 + Src1) * C0), C0) + C1 - C1) * C2,
        reference=lambda in0, in1, s0, s1, imm2: _q(
            (
                (
                    in0.astype(np.float32).reshape(in0.shape[0], -1)
                    + in1.astype(np.float32).reshape(in1.shape[0], -1)
                ).astype(np.float32)
                * np.float32(s0)
            ).astype(np.float32),
            s0, s1, imm2,
        ).reshape(in0.shape),
    ),
)

N_CORES = 8
P = 128          # channels == partitions
NIMG = 8         # images per core
H = W = 28
HP = 30          # padded width / logical padded height
HR = 31          # allocated rows per image (junk-run overflow row)
HF = 14          # rows per chunk
NCH = NIMG * 2   # chunks per core
NRUN = HF * HP   # 420: moving-operand run per tap
MAGIC = float(2 ** 23)
F32 = mybir.dt.float32
F16 = mybir.dt.float16
BF16 = mybir.dt.bfloat16
AF = mybir.ActivationFunctionType
OP = mybir.AluOpType

CONV_GROUP = 3   # psum tiles in flight per conv group


def _quant_int(w: np.ndarray) -> np.ndarray:
    """DoReFa 4-bit weight quantization -> integer numerators m (wq = m/15)."""
    t = np.tanh(w.astype(np.float32))
    mx = np.max(np.abs(t))
    tq = t / (np.float32(2.0) * mx) + np.float32(0.5)
    j = np.round(tq * np.float32(15.0))
    return (np.float32(2.0) * j - np.float32(15.0)).astype(np.float32)


def _weights_lhsT(m: np.ndarray, dtype) -> np.ndarray:
    """[o,i,ky,kx] integer weights -> lhsT layout [i, tap, o]."""
    return np.ascontiguousarray(m.transpose(1, 2, 3, 0).reshape(P, 9, P)).astype(dtype)


def _emit(nc, tc):
    x_d = nc.dram_tensor("x", [NIMG, P, H, W], F32, kind="ExternalInput").ap()
    w1_d = nc.dram_tensor("wq1", [P, 9, P], F16, kind="ExternalInput").ap()
    w2_d = nc.dram_tensor("wq2", [P, 9, P], BF16, kind="ExternalInput").ap()
    gb_d = nc.dram_tensor("gb", [P, 4], F32, kind="ExternalInput").ap()
    eye_d = nc.dram_tensor("eye", [P, P], F32, kind="ExternalInput").ap()
    out_d = nc.dram_tensor("out", [NIMG, P, H, W], F32, kind="ExternalOutput").ap()
    wu_d = nc.dram_tensor("wu", [P], F32, kind="ExternalOutput").ap()

    rg = [list(range(N_CORES))]

    with (
        tc.tile_pool(name="persist", bufs=1) as pp,
        tc.tile_pool(name="rot", bufs=2) as rp,
        tc.tile_pool(name="fin", bufs=4) as fp,
        tc.tile_pool(name="cpsum", bufs=6, space="PSUM") as pcp,
        tc.tile_pool(name="tpsum", bufs=1, space="PSUM") as tps,
        tc.tile_pool(name="dram", bufs=1, space="DRAM") as dp,
    ):
        # ---- warmup collective: absorb ncfw first-call + core-skew cost ----
        wu_in = dp.tile([2, P], F32, tag="wuin", name="wuin")
        wu_out = dp.tile([N_CORES * 2, P], F32, tag="wuout", name="wuout")
        nc.gpsimd.dma_start(out=wu_in.opt(), in_=gb_d[:, 0:2])
        nc.gpsimd.collective_compute(
            "AllGather", OP.bypass, replica_groups=rg,
            ins=[wu_in.opt()], outs=[wu_out.opt()],
        )

        xpad = pp.tile([P, NIMG, HP, HP], F32, tag="xpad")
        xh = pp.tile([P, NIMG, HR, HP], F16, tag="xh")
        a1 = pp.tile([P, NIMG, HR, HP], BF16, tag="a1")
        raw1 = pp.tile([P, NIMG, H, W], F32, tag="raw1")
        raw2 = pp.tile([P, NIMG, H, W], F32, tag="raw2")
        w1s = pp.tile([P, 9, P], F16, tag="w1s")
        w2s = pp.tile([P, 9, P], BF16, tag="w2s")
        gbs = pp.tile([P, 4], F32, tag="gbs")
        eyes = pp.tile([P, P], F32, tag="eyes")

        # ---- zero padding borders ----
        nc.vector.memset(xpad[:, :, 0, :], 0.0)
        nc.vector.memset(xpad[:, :, HP - 1, :], 0.0)
        nc.vector.memset(xpad[:, :, 1 : HP - 1, 0], 0.0)
        nc.vector.memset(xpad[:, :, 1 : HP - 1, HP - 1], 0.0)
        for t in (xh, a1):  # row 30 is junk-run overflow: must be finite
            nc.vector.memset(t[:, :, HR - 1, :], 0.0)
        nc.vector.memset(a1[:, :, 0, :], 0.0)
        nc.vector.memset(a1[:, :, HP - 1, :], 0.0)
        nc.vector.memset(a1[:, :, 1 : HP - 1, 0], 0.0)
        nc.vector.memset(a1[:, :, 1 : HP - 1, HP - 1], 0.0)

        # ---- stream x in (two DMA queues); round to fp16 per image ----
        for n in range(NIMG):
            q = nc.sync if n % 2 == 0 else nc.scalar
            q.dma_start(out=xpad[:, n, 1 : 1 + H, 1 : 1 + W], in_=x_d[n])
            nc.vector.tensor_copy(out=xh[:, n, 0:HP, :], in_=xpad[:, n])
            if n == 1:
                nc.scalar.dma_start(out=w1s[:], in_=w1_d)
        nc.sync.dma_start(out=w2s[:], in_=w2_d)
        nc.sync.dma_start(out=gbs[:], in_=gb_d)
        nc.sync.dma_start(out=eyes[:], in_=eye_d)

        # pre-load the scalar engine's Sqrt activation table while conv1 runs:
        # the first Sqrt otherwise pays a ~1.3us ACT_TABLE_LOAD right on the
        # post-AllReduce critical path.
        warm = pp.tile([P, 1], F32, tag="warm")
        nc.vector.memset(warm[:], 1.0)
        nc.scalar.activation(out=warm[:], in_=warm[:], func=AF.Sqrt)

        def conv(pieces, wsb, rawbuf, stbuf):
            """9-tap shifted conv, contiguous 420-elem moving operands.
            PSUM tile is [P, 14, 30]; columns 28/29 are junk. ACT copies the
            real columns to SBUF; DVE bn_stats accumulates per-row stats."""
            flats = {}
            for pi, piece in enumerate(pieces):
                for n in range(NIMG):
                    flats[(pi, n)] = piece[:, n].rearrange("p h w -> p (h w)")
            nmm = 9 * len(pieces)
            # Small leading groups so the PE starts as soon as image 0's
            # input is ready, instead of waiting for images 0 AND 1.
            groups = [[0], [1, 2]] + [
                list(range(gs, min(gs + CONV_GROUP, NCH)))
                for gs in range(3, NCH, CONV_GROUP)
            ]
            for grp in groups:
                pt = {
                    ci: pcp.tile([P, HF, HP], F32, tag="cps", name=f"cps{ci}")
                    for ci in grp
                }
                for t in range(9):
                    dy, dx = divmod(t, 3)
                    for pi in range(len(pieces)):
                        k = t * len(pieces) + pi
                        for ci in grp:
                            n, hh = divmod(ci, 2)
                            off = (hh * HF + dy) * HP + dx
                            nc.tensor.matmul(
                                pt[ci][:],
                                wsb[:, t, :],
                                flats[(pi, n)][:, off : off + NRUN],
                                start=(k == 0),
                                stop=(k == nmm - 1),
                            )
                for ci in grp:
                    n, hh = divmod(ci, 2)
                    h0 = hh * HF
                    if ci == NCH - 1:
                        # last chunk: copy on the (idle) DVE instead of the
                        # backlogged scalar queue -- its PSUM copy gates the
                        # stats collective trigger.
                        nc.vector.tensor_copy(
                            out=rawbuf[:, n, h0 : h0 + HF, :],
                            in_=pt[ci][:, :, 0:W],
                        )
                    else:
                        nc.scalar.activation(
                            out=rawbuf[:, n, h0 : h0 + HF, :],
                            in_=pt[ci][:, :, 0:W],
                            func=AF.Copy,
                        )
                    nc.vector.bn_stats(
                        out=stbuf[:, 6 * ci : 6 * (ci + 1)],
                        in_=rawbuf[:, n, h0 : h0 + HF, :].rearrange("p h w -> p (h w)"),
                    )

        def bn_scalars(ph, stbuf, c_mean, c_ey2, g_col, b_col, fold_scale):
            """Cross-core stat AllReduce + BN affine coefficients.

            Per-core (mean, E[y^2]) is PE-transposed to a [2,P] layout so the
            1KB each-way DMAs are 2 big packets instead of 128 tiny ones,
            AllReduce(add)'d across cores, and transposed back into PSUM.
            c_mean/c_ey2 fold the 1/(n_cores*scale) normalization into the
            scalar math; BN_EPS is dropped (vars here are O(1e2..1e5), the
            1e-5 eps is ~1e-9 relative).

            Returns (scaleA, biasB) with
              scaleA = rsqrt(var)*gamma * fold_scale   (raw -> bn domain)
              biasB  = beta - mean*rsqrt(var)*gamma
            """

            def vt(tag):
                return pp.tile([P, 1], F32, tag=f"{tag}{ph}", name=f"{tag}{ph}")

            cpk = pp.tile([P, 2], F32, tag=f"cpk{ph}", name=f"cpk{ph}")
            m2l = vt("m2l")
            nc.vector.bn_aggr(out=cpk[:], in_=stbuf[:])
            nc.vector.tensor_mul(out=m2l[:], in0=cpk[:, 0:1], in1=cpk[:, 0:1])
            nc.vector.tensor_add(out=cpk[:, 1:2], in0=cpk[:, 1:2], in1=m2l[:])
            # transpose [P,2] -> [2,P] on the (idle) PE, copy to SBUF, 1 DMA
            t2p = tps.tile([2, P], F32, tag="t2p", name=f"t2p{ph}")
            nc.tensor.transpose(t2p[:], cpk[:], eyes[:])
            c2s = pp.tile([2, P], F32, tag=f"c2s{ph}", name=f"c2s{ph}")
            nc.scalar.activation(out=c2s[:], in_=t2p[:], func=AF.Copy)
            cin = dp.tile([2, P], F32, tag=f"cin{ph}", name=f"cin{ph}")
            cout = dp.tile([2, P], F32, tag=f"cout{ph}", name=f"cout{ph}")
            nc.sync.dma_start(out=cin[:], in_=c2s[:])
            if ph == 1:
                # Hold this trigger until the warmup collective has fully
                # completed: a trigger that lands while ncfw is busy pays a
                # ~16us slow re-arm, one landing on an idle ncfw ~0.3us. The
                # wu_out-consuming store blocks the gpsimd queue (and only
                # it) until the warmup AllGather is done.
                nc.gpsimd.dma_start(out=wu_d, in_=wu_out[0, :])
            nc.gpsimd.collective_compute(
                "AllReduce", OP.add, replica_groups=rg,
                ins=[cin.opt()], outs=[cout.opt()],
            )
            c2p = pp.tile([2, P], F32, tag=f"c2p{ph}", name=f"c2p{ph}")
            nc.sync.dma_start(out=c2p[:], in_=cout[:])
            stp = tps.tile([P, 2], F32, tag="stp", name=f"stp{ph}")
            nc.tensor.transpose(stp[:], c2p[:], eyes[0:2, 0:2])

            mn, m2, u, s, r = vt("mn"), vt("m2"), vt("u"), vt("s"), vt("r")
            # mean = c_mean * sum; mean^2 on DVE (no ACT Square table switch)
            nc.scalar.activation(out=mn[:], in_=stp[:, 0:1], func=AF.Copy, scale=c_mean)
            nc.vector.tensor_mul(out=m2[:], in0=mn[:], in1=mn[:])
            # u = var = E[y^2] - mean^2
            nc.vector.scalar_tensor_tensor(
                out=u[:], in0=stp[:, 1:2], scalar=c_ey2, in1=m2[:],
                op0=OP.mult, op1=OP.subtract,
            )
            nc.scalar.activation(out=s[:], in_=u[:], func=AF.Sqrt)
            nc.vector.reciprocal(out=r[:], in_=s[:])
            rgm, scaleA, b0, biasB = vt("rg"), vt("sA"), vt("b0"), vt("bB")
            nc.vector.tensor_mul(out=rgm[:], in0=r[:], in1=gbs[:, g_col : g_col + 1])
            if fold_scale == 1.0:
                scaleA = rgm
            else:
                nc.vector.tensor_scalar(
                    out=scaleA[:], in0=rgm[:], scalar1=fold_scale, scalar2=None, op0=OP.mult
                )
            nc.vector.tensor_mul(out=b0[:], in0=mn[:], in1=rgm[:])
            nc.vector.tensor_sub(
                out=biasB[:], in0=gbs[:, b_col : b_col + 1], in1=b0[:]
            )
            return scaleA, biasB

        # ================= phase 1: conv1 + BN1 stats =================
        st1 = pp.tile([P, NCH * 6], F32, tag="st1")
        conv([xh], w1s, raw1, st1)
        sA1, bB1 = bn_scalars(
            1, st1, 1.0 / (N_CORES * 15.0), 1.0 / (N_CORES * 225.0), 0, 1, 1.0 / 15.0
        )

        # ============ act1 quantization -> integers in bf16 (per image) ============
        # image 0 is processed in two row-bands so conv2's first chunk (which
        # needs only padded rows 0..15) can start before the whole image is
        # quantized -- this sits on the serial post-collective path.
        for n in range(NIMG):
            bands = ((0, 16), (16, H)) if n <= 1 else ((0, H),)
            for r0, r1 in bands:
                u = rp.tile([P, r1 - r0, W], F32, tag="uq", name=f"uq{n}_{r0}")
                nc.scalar.activation(
                    out=u[:], in_=raw1[:, n, r0:r1, :], func=AF.Relu,
                    bias=bB1[:], scale=sA1[:],
                )
                nc.vector._custom_dve(
                    QUANT_OP,
                    out=a1[:, n, 1 + r0 : 1 + r1, 1 : 1 + W],
                    in0=u[:],
                    s0=15.0,
                    s1=MAGIC,
                    imm2=1.0,
                )

        # ================= phase 2: conv2 + BN2 stats =================
        st2 = pp.tile([P, NCH * 6], F32, tag="st2")
        conv([a1], w2s, raw2, st2)
        sA2, bB2 = bn_scalars(
            2, st2, 1.0 / (N_CORES * 225.0), 1.0 / (N_CORES * 225.0 * 225.0), 2, 3, 1.0 / 225.0
        )

        # ========== final: bn2 + residual + qact ==========
        # ACT applies the BN affine two images at a time (halves the per-op
        # overhead; the per-image DVE quant becomes the pacer), one fused DVE
        # op per image does residual add + clip + round + rescale; output
        # DMAs rotate over three queues, and the last image's store is split
        # across two queues to shorten the tail.
        for b0, b1 in ((0, 1), (1, 3), (3, 5), (5, 7), (7, 8)):
            nb = b1 - b0
            p1 = fp.tile([P, nb * H * W], F32, tag="p1", name=f"p1_{b0}")
            nc.scalar.activation(
                out=p1[:],
                in_=raw2[:, b0:b1].rearrange("p n h w -> p (n h w)"),
                func=AF.Identity,
                bias=bB2[:],
                scale=sA2[:],
            )
            for k in range(nb):
                n = b0 + k
                og = fp.tile([P, H * W], F32, tag="og", name=f"og_{n}")
                nc.vector._custom_dve(
                    QUANT_RES_OP,
                    out=og[:],
                    in0=xpad[:, n, 1 : 1 + H, 1 : 1 + W],
                    in1=p1[:, k * H * W : (k + 1) * H * W],
                    s0=15.0,
                    s1=MAGIC,
                    imm2=1.0 / 15.0,
                )
                if n == NIMG - 1:
                    half = (H // 2) * W
                    nc.sync.dma_start(out=out_d[n, :, 0 : H // 2, :], in_=og[:, 0:half])
                    nc.scalar.dma_start(out=out_d[n, :, H // 2 :, :], in_=og[:, half:])
                else:
                    q = (nc.sync, nc.gpsimd, nc.scalar)[n % 3]
                    q.dma_start(out=out_d[n], in_=og[:])


_PROGRAM = None


def get_program():
    global _PROGRAM
    if _PROGRAM is None:
        nc = bacc.Bacc(
            "TRN2",
            target_bir_lowering=False,
            debug=False,
            enable_asserts=True,
            num_devices=N_CORES,
        )
        with tile.TileContext(nc, num_cores=N_CORES) as tc:
            _emit(nc, tc)
        nc.compile()
        _PROGRAM = nc
    return _PROGRAM


def make_in_maps(inputs):
    x = np.asarray(inputs["x"], np.float32)
    m1 = _quant_int(np.asarray(inputs["w1"], np.float32))
    mask = (np.asarray(inputs["mask2"], np.float32) > 0.5).astype(np.float32)
    m2 = _quant_int(np.asarray(inputs["w2"], np.float32) * mask)
    wq1 = _weights_lhsT(m1, np.float16)
    wq2 = _weights_lhsT(m2, ml_dtypes.bfloat16)
    gb = np.stack(
        [
            np.asarray(inputs["gamma1"], np.float32),
            np.asarray(inputs["beta1"], np.float32),
            np.asarray(inputs["gamma2"], np.float32),
            np.asarray(inputs["beta2"], np.float32),
        ],
        axis=1,
    )
    gb = np.ascontiguousarray(gb)
    eye = np.eye(P, dtype=np.float32)
    return [
        {
            "x": np.ascontiguousarray(x[NIMG * i : NIMG * (i + 1)]),
            "wq1": wq1,
            "wq2": wq2,
            "gb": gb,
            "eye": eye,
        }
        for i in range(N_CORES)
    ]


def run(inputs, **kwargs) -> bass_utils.BassKernelResults:
    nc = get_program()
    return bass_utils.run_bass_kernel_spmd(
        nc, make_in_maps(inputs), core_ids=list(range(N_CORES)), **kwargs
    )


def kernel(**inputs) -> np.ndarray:
    res = run(inputs)
    return np.concatenate(
        [res.results[i]["out"] for i in range(N_CORES)], axis=0
    ).astype(np.float32)
